# revision 11
# baseline (speedup 1.0000x reference)
"""Trainium2 Bass kernel for nn_ClauseToVarLayer (GNN clause->var message passing + LSTM cell).

reference:
    msg = segment_sum(x_c[edge_clause], edge_var, num_segments=n_vars)
    gates = msg @ W_ih.T + b_ih + h @ W_hh.T + b_hh
    i, f, g, o = split(gates); c' = sig(f)*c + sig(i)*tanh(g); h' = sig(o)*tanh(c')

Sharding: 1D partition by var id across 8 cores (12500 vars each, padded to
98 tiles of 128). Each core receives the full clause-feature table (gather
source), its own edge tables, its h/c slices, and replicated LSTM weights.
No cross-core communication.

The edge gather uses the CounterMachine `dma_gather` SWDGE primitive (high
descriptor rate), which takes int16 indices: clauses are assigned to 13
banks of <=32768 rows by a host-side balancing pass that guarantees every
(var-tile, bank) cell has <=128 edges. x_c is shipped permuted into bank
order as bf16 with each row duplicated ([row row], 512B), so gather
descriptors are 512B (full DMA efficiency) and the bf16 operand needs no
on-chip cast.

Per var-tile of 128 vars (13 chunks of 128 edge slots, one per bank):
  1. dma_gather per (super-group of 8 tiles, bank) -> E_b [128 slots(part),
     8 tiles, 256] bf16
  2. DVE: selector sel[p, b, v] = (evrel[p, b] == v) via broadcast is_equal
  3. TensorE: msgT[feat, var] accumulated over 13 chunk matmuls in PSUM
  4. LSTM gates = ones.T@bias + msgT.T@W_ihT + hT.T@W_hhT as float32r
     matmuls (full PE rate at N=512, ~fp32 precision)
  5. ScalarE activations; VectorE elementwise batched across groups.
"""

import os
import sys

import numpy as np

for _p in ("/opt/trn_rl_repo",):
    if _p not in sys.path and os.path.isdir(_p):
        sys.path.insert(0, _p)

import ml_dtypes

import concourse.bacc as bacc
import concourse.bass as bass
import concourse.mybir as mybir
from concourse.bass_utils import run_bass_kernel_spmd
from concourse.tile import TileContext

F32 = mybir.dt.float32
F32R = mybir.dt.float32r
BF16 = mybir.dt.bfloat16
I16 = mybir.dt.int16
AF = mybir.ActivationFunctionType

N_CLAUSES, N_VARS, N_EDGES, D = 400000, 100000, 1200000, 128
N_CORES = 8
VPC = N_VARS // N_CORES          # 12500 vars per core
VT = 128                         # vars per tile
NT = (VPC + VT - 1) // VT        # 98 var tiles per core
VPAD = NT * VT                   # 12544 padded vars per core
NB = 13                          # clause banks (each <= 32768 rows, int16)
BANK_CAP = 32768
CELL_CAP = 128                   # max edges per (var-tile, bank) cell
SG = 8                           # var-tiles per gather super-group
GRP = 4                          # var-tiles per elementwise/LSTM group
SENTINEL = 255.0                 # evrel value that never matches iota (0..127)

LAST_RESULTS = None


# --------------------------------------------------------------------------
# host-side prep
# --------------------------------------------------------------------------

def _assign_banks(ec, tile_of_edge, n_tiles_total, rng):
    """Assign each clause to one of NB banks s.t. every (tile, bank) cell has
    <= CELL_CAP edges and every bank has <= BANK_CAP clauses."""
    n_cells = n_tiles_total * NB
    bank = rng.integers(0, NB, N_CLAUSES).astype(np.int64)

    # clause -> its edges' tiles, padded matrix [N_CLAUSES, maxdeg]
    order = np.argsort(ec, kind="stable")
    ec_s = ec[order]
    til_s = tile_of_edge[order]
    counts = np.bincount(ec_s, minlength=N_CLAUSES)
    maxdeg = int(counts.max())
    starts = np.zeros(N_CLAUSES + 1, np.int64)
    np.cumsum(counts, out=starts[1:])
    tile_mat = np.full((N_CLAUSES, maxdeg), -1, np.int64)
    pos = np.arange(len(ec_s)) - starts[ec_s]
    tile_mat[ec_s, pos] = til_s
    valid = tile_mat >= 0
    tile_mat_c = np.where(valid, tile_mat, 0)

    for it in range(300):
        cell = tile_of_edge * NB + bank[ec]
        load = np.bincount(cell, minlength=n_cells)
        if load.max() <= CELL_CAP:
            break
        # overflow tail: within each cell, edges ranked >= CELL_CAP must move
        eorder = np.argsort(cell, kind="stable")
        cs = cell[eorder]
        cstart = np.zeros(n_cells + 1, np.int64)
        np.cumsum(load, out=cstart[1:])
        rank = np.arange(len(cs)) - cstart[cs]
        overflow_edges = eorder[rank >= CELL_CAP]
        cand = np.unique(ec[overflow_edges])
        # per candidate clause: for each bank b, max load over its tiles' cells
        cl = load.reshape(n_tiles_total, NB)
        tl = cl[tile_mat_c[cand]]                   # [n_cand, maxdeg, NB]
        tl = np.where(valid[cand][:, :, None], tl, 0)
        worst = tl.max(axis=1).astype(np.float64)   # [n_cand, NB]
        # banks already at/over cap for any of the clause's tiles are barred
        worst[worst >= CELL_CAP] = 1e9
        score = worst + rng.random(worst.shape)
        newb = np.argmin(score, axis=1)
        bank[cand] = newb
    else:
        raise RuntimeError("bank balancing did not converge (cells)")

    # enforce bank row caps by moving random clauses out of oversized banks
    for it in range(100):
        sizes = np.bincount(bank, minlength=NB)
        if (sizes <= BANK_CAP).all():
            break
        src = int(np.argmax(sizes))
        excess = int(sizes[src] - BANK_CAP)
        members = np.where(bank == src)[0]
        movers = rng.choice(members, size=excess * 2, replace=False)
        dst = int(np.argmin(sizes))
        bank[movers] = dst
    else:
        raise RuntimeError("bank balancing did not converge (sizes)")

    # final verification of cell caps (size moves may rarely break them)
    cell = tile_of_edge * NB + bank[ec]
    load = np.bincount(cell, minlength=n_cells)
    if load.max() > CELL_CAP:
        raise RuntimeError("cell cap violated after size repair")
    return bank


def host_prep(edge_clause, edge_var, x_c, seed=0):
    """Returns (xdup [R,256] bf16 bank-permuted+duplicated, bank_base [NB+1],
    idx16 [8, n_sg, NB, 128, SG*8] int16, evrel [8, 128, NT*NB] bf16)."""
    ec = np.asarray(edge_clause, dtype=np.int64)
    ev = np.asarray(edge_var, dtype=np.int64)
    x_c = np.asarray(x_c, dtype=np.float32)
    rng = np.random.default_rng(seed)

    core = ev // VPC
    tloc = (ev - core * VPC) // VT
    tile_of_edge = core * NT + tloc
    bank = _assign_banks(ec, tile_of_edge, N_CORES * NT, rng)

    # permute clauses into bank-major order
    sizes = np.bincount(bank, minlength=NB)
    bank_base = np.zeros(NB + 1, np.int64)
    np.cumsum(sizes, out=bank_base[1:])
    perm_order = np.argsort(bank, kind="stable")     # new_row -> old clause
    new_row = np.empty(N_CLAUSES, np.int64)
    new_row[perm_order] = np.arange(N_CLAUSES)       # old clause -> new row
    local_row = new_row - bank_base[bank]            # int16-safe (< 32768)
    assert local_row.min() >= 0 and local_row.max() < BANK_CAP

    xb = x_c.astype(ml_dtypes.bfloat16)
    xdup = np.empty((N_CLAUSES, 2 * D), ml_dtypes.bfloat16)
    xdup[:, :D] = xb[perm_order]
    xdup[:, D:] = xb[perm_order]

    n_sg = (NT + SG - 1) // SG
    idx16 = np.zeros((N_CORES, n_sg, 128, NB, SG * 8), np.int16)
    evrel = np.full((N_CORES, 128, NT * NB), SENTINEL, np.float32)

    ev_local = ev - core * VPC
    for k in range(N_CORES):
        m = core == k
        eck, evk = ec[m], ev_local[m]
        o = np.argsort(evk, kind="stable")
        eck, evk = eck[o], evk[o]
        bk = bank[eck]
        lr = local_row[eck]
        # order by (tile, bank) then pack slots
        tk = evk // VT
        o2 = np.lexsort((bk, tk))
        eck, evk, bk, lr, tk = eck[o2], evk[o2], bk[o2], lr[o2], tk[o2]
        cell_ids = tk * NB + bk
        cell_counts = np.bincount(cell_ids, minlength=NT * NB)
        assert cell_counts.max() <= CELL_CAP
        cb = np.zeros(NT * NB + 1, np.int64)
        np.cumsum(cell_counts, out=cb[1:])
        slot_in_cell = np.arange(len(eck)) - cb[cell_ids]
        # slot tables [NT, NB, 128]
        idx_tab = np.zeros((NT, NB, CELL_CAP), np.int64)
        evr_tab = np.full((NT, NB, CELL_CAP), SENTINEL, np.float32)
        idx_tab[tk, bk, slot_in_cell] = lr
        evr_tab[tk, bk, slot_in_cell] = evk - tk * VT
        # pad slots repeat a valid row (cheap HBM re-read), evrel stays sentinel
        for t in range(NT):
            for b in range(NB):
                n = int(cell_counts[t * NB + b])
                if n < CELL_CAP:
                    fill = idx_tab[t, b, n - 1] if n > 0 else 0
                    idx_tab[t, b, n:] = fill
        # evrel layout [128 slots(part), (t, b)]
        evrel[k] = evr_tab.transpose(2, 0, 1).reshape(128, NT * NB)
        # idx16 per call (sg, b): logical n = (t_rel*128 + p) -> [n%16, n//16]
        for s in range(n_sg):
            t0, t1 = s * SG, min((s + 1) * SG, NT)
            for b in range(NB):
                seq = idx_tab[t0:t1, b, :].reshape(-1)      # [(t1-t0)*128]
                w = np.zeros((16, SG * 8), np.int64)
                n = len(seq)
                # wrap: logical n -> [n % 16, n // 16]
                w[np.arange(n) % 16, np.arange(n) // 16] = seq
                idx16[k, s, :, b, :] = np.tile(w, (8, 1)).astype(np.int16)
    return xdup, bank_base, idx16, evrel.astype(ml_dtypes.bfloat16)


# --------------------------------------------------------------------------
# device program
# --------------------------------------------------------------------------

def build_program(bank_sizes, nt=NT, sg=SG, grp=GRP, num_devices=N_CORES,
                  n_rows=N_CLAUSES, repeat=1):
    """bank_sizes: rows per bank (len NB). Same for all cores."""
    vpad = nt * VT
    n_sg = (nt + sg - 1) // sg
    bank_base = np.zeros(NB + 1, np.int64)
    np.cumsum(np.asarray(bank_sizes, np.int64), out=bank_base[1:])
    assert bank_base[-1] == n_rows

    nc = bacc.Bacc("TRN2", target_bir_lowering=False, debug=False,
                   num_devices=num_devices)

    xdup = nc.dram_tensor("xdup", [n_rows, 2 * D], BF16, kind="ExternalInput")
    idx_t = nc.dram_tensor("idx16", [n_sg, 128, NB, sg * 8], I16,
                           kind="ExternalInput")
    evr_t = nc.dram_tensor("evrel", [128, nt * NB], BF16, kind="ExternalInput")
    hT_t = nc.dram_tensor("hT", [D, vpad], F32R, kind="ExternalInput")
    c_t = nc.dram_tensor("c_in", [vpad, D], F32, kind="ExternalInput")
    wih_t = nc.dram_tensor("wihT", [D, 4 * D], F32R, kind="ExternalInput")
    whh_t = nc.dram_tensor("whhT", [D, 4 * D], F32R, kind="ExternalInput")
    bias_t = nc.dram_tensor("bias2", [1, 4 * D], F32R, kind="ExternalInput")
    ones_t = nc.dram_tensor("ones", [1, VT], F32R, kind="ExternalInput")
    iota_t = nc.dram_tensor("iota", [128, VT], BF16, kind="ExternalInput")
    h_out = nc.dram_tensor("h_out", [vpad, D], F32, kind="ExternalOutput")
    c_out = nc.dram_tensor("c_out", [vpad, D], F32, kind="ExternalOutput")

    with TileContext(nc) as tc:
        with (
            tc.tile_pool(name="const", bufs=1) as constp,
            tc.tile_pool(name="idxp", bufs=2) as idxp,
            tc.tile_pool(name="edges", bufs=2) as edgep,
            tc.tile_pool(name="sel", bufs=2) as selp,
            tc.tile_pool(name="msg", bufs=2) as msgp,
            tc.tile_pool(name="io", bufs=2) as iop,
            tc.tile_pool(name="ifgo", bufs=2) as actp,
            tc.tile_pool(name="res", bufs=2) as resp,
            tc.tile_pool(name="psm", bufs=2, space="PSUM") as psmm,
            tc.tile_pool(name="psg", bufs=4, space="PSUM") as psgg,
        ):
            evr_sb = constp.tile([128, nt * NB], BF16)
            nc.sync.dma_start(out=evr_sb[:], in_=evr_t[:])
            wih_sb = constp.tile([D, 4 * D], F32R)
            nc.sync.dma_start(out=wih_sb[:], in_=wih_t[:])
            whh_sb = constp.tile([D, 4 * D], F32R)
            nc.sync.dma_start(out=whh_sb[:], in_=whh_t[:])
            bias_sb = constp.tile([1, 4 * D], F32R)
            nc.sync.dma_start(out=bias_sb[:], in_=bias_t[:])
            ones_sb = constp.tile([1, VT], F32R)
            nc.sync.dma_start(out=ones_sb[:], in_=ones_t[:])
            iota_sb = constp.tile([128, VT], BF16)
            nc.sync.dma_start(out=iota_sb[:], in_=iota_t[:])

            for _rep in range(repeat):
              for s in range(n_sg):
                t0 = s * sg
                sgn = min(sg, nt - t0)
                # load this super-group's int16 index tables + gather banks
                idx_sb = idxp.tile([128, NB, sg * 8], I16, tag="idx")
                nc.sync.dma_start(out=idx_sb[:], in_=idx_t[s])
                ebs = []
                for b in range(NB):
                    eb = edgep.tile([128, sg, 2 * D], BF16, tag=f"E{b}")
                    nc.gpsimd.dma_gather(
                        out_ap=eb[:, :sgn, :],
                        in_ap=xdup[int(bank_base[b]) : int(bank_base[b + 1]), :],
                        idxs_ap=idx_sb[:, b, : sgn * 8],
                        num_idxs=sgn * 128,
                        num_idxs_reg=sgn * 128,
                        elem_size=2 * D,
                    )
                    ebs.append(eb)

                for j0 in range(0, sgn, grp):
                    g_n = min(grp, sgn - j0)
                    r0 = (t0 + j0) * VT
                    r1 = r0 + g_n * VT
                    hT_g = iop.tile([128, grp * VT], F32R, tag="hTg")
                    nc.sync.dma_start(out=hT_g[:, : g_n * VT],
                                      in_=hT_t[:, r0:r1])
                    c_g = iop.tile([128, grp, D], F32, tag="cg")
                    nc.sync.dma_start(
                        out=c_g[:, :g_n, :],
                        in_=c_t[r0:r1, :].rearrange("(g p) d -> p g d", p=128),
                    )
                    ifgo = actp.tile([128, grp, 4 * D], F32)

                    for j in range(g_n):
                        t = t0 + j0 + j
                        trel = j0 + j
                        e0 = t * NB
                        sel = selp.tile([128, NB, VT], BF16, tag="sel")
                        nc.vector.tensor_tensor(
                            out=sel[:],
                            in0=evr_sb[:, e0 : e0 + NB]
                            .rearrange("p (g o) -> p g o", o=1)
                            .to_broadcast([128, NB, VT]),
                            in1=iota_sb[:]
                            .rearrange("p (g v) -> p g v", g=1)
                            .to_broadcast([128, NB, VT]),
                            op=mybir.AluOpType.is_equal,
                        )
                        ps = psmm.tile([128, VT], F32)
                        for b in range(NB):
                            nc.tensor.matmul(
                                out=ps[:],
                                lhsT=ebs[b][:, trel, 0:D],
                                rhs=sel[:, b, :],
                                start=(b == 0),
                                stop=(b == NB - 1),
                            )
                        msgT = msgp.tile([128, VT], F32R)
                        nc.scalar.activation(out=msgT[:], in_=ps[:],
                                             func=AF.Copy)
                        gates = psgg.tile([VT, 4 * D], F32)
                        nc.tensor.matmul(out=gates[:], lhsT=ones_sb[:],
                                         rhs=bias_sb[:], start=True, stop=False)
                        nc.tensor.matmul(out=gates[:], lhsT=msgT[:],
                                         rhs=wih_sb[:], start=False, stop=False)
                        nc.tensor.matmul(
                            out=gates[:],
                            lhsT=hT_g[:, j * VT : (j + 1) * VT],
                            rhs=whh_sb[:], start=False, stop=True,
                        )
                        nc.scalar.activation(out=ifgo[:, j, 0 : 2 * D],
                                             in_=gates[:, 0 : 2 * D],
                                             func=AF.Sigmoid)
                        nc.scalar.activation(out=ifgo[:, j, 2 * D : 3 * D],
                                             in_=gates[:, 2 * D : 3 * D],
                                             func=AF.Tanh)
                        nc.scalar.activation(out=ifgo[:, j, 3 * D : 4 * D],
                                             in_=gates[:, 3 * D : 4 * D],
                                             func=AF.Sigmoid)

                    i_ = ifgo[:, :g_n, 0:D]
                    f_ = ifgo[:, :g_n, D : 2 * D]
                    g_ = ifgo[:, :g_n, 2 * D : 3 * D]
                    o_ = ifgo[:, :g_n, 3 * D : 4 * D]
                    t1_ = resp.tile([128, grp, D], F32, tag="t1")
                    nc.vector.tensor_mul(out=t1_[:, :g_n, :], in0=f_,
                                         in1=c_g[:, :g_n, :])
                    t2_ = resp.tile([128, grp, D], F32, tag="t2")
                    nc.vector.tensor_mul(out=t2_[:, :g_n, :], in0=i_, in1=g_)
                    cn = resp.tile([128, grp, D], F32, tag="cn")
                    nc.vector.tensor_add(out=cn[:, :g_n, :],
                                         in0=t1_[:, :g_n, :],
                                         in1=t2_[:, :g_n, :])
                    th = resp.tile([128, grp, D], F32, tag="th")
                    nc.scalar.activation(out=th[:, :g_n, :],
                                         in_=cn[:, :g_n, :], func=AF.Tanh)
                    hn = resp.tile([128, grp, D], F32, tag="hn")
                    nc.vector.tensor_mul(out=hn[:, :g_n, :], in0=o_,
                                         in1=th[:, :g_n, :])
                    nc.sync.dma_start(
                        out=c_out[r0:r1, :].rearrange("(g p) d -> p g d", p=128),
                        in_=cn[:, :g_n, :],
                    )
                    nc.sync.dma_start(
                        out=h_out[r0:r1, :].rearrange("(g p) d -> p g d", p=128),
                        in_=hn[:, :g_n, :],
                    )

    nc.compile()
    return nc


# --------------------------------------------------------------------------
# entry point
# --------------------------------------------------------------------------

def kernel(edge_clause, edge_var, x_c, h, c, W_ih, W_hh, b_ih, b_hh):
    global LAST_RESULTS
    xdup, bank_base, idx16, evrel = host_prep(edge_clause, edge_var, x_c)
    bank_sizes = np.diff(bank_base)

    h = np.asarray(h, dtype=np.float32)
    c = np.asarray(c, dtype=np.float32)
    wihT = np.ascontiguousarray(np.asarray(W_ih, np.float32).T)
    whhT = np.ascontiguousarray(np.asarray(W_hh, np.float32).T)
    bias2 = (np.asarray(b_ih, np.float32) + np.asarray(b_hh, np.float32))[None, :]
    ones = np.ones((1, VT), np.float32)
    iota_bf = np.ascontiguousarray(
        np.broadcast_to(np.arange(VT, dtype=np.float32), (128, VT)).astype(
            ml_dtypes.bfloat16
        )
    )

    nc = build_program(bank_sizes)

    in_maps = []
    for k in range(N_CORES):
        hT_k = np.zeros((D, VPAD), np.float32)
        hT_k[:, :VPC] = h[k * VPC : (k + 1) * VPC].T
        c_k = np.zeros((VPAD, D), np.float32)
        c_k[:VPC] = c[k * VPC : (k + 1) * VPC]
        in_maps.append(
            {
                "xdup": xdup,
                "idx16": idx16[k],
                "evrel": evrel[k],
                "hT": hT_k,
                "c_in": c_k,
                "wihT": wihT,
                "whhT": whhT,
                "bias2": bias2,
                "ones": ones,
                "iota": iota_bf,
            }
        )

    trace = bool(int(os.environ.get("KBENCH_TRACE", "0")))
    res = run_bass_kernel_spmd(
        nc, in_maps, core_ids=list(range(N_CORES)), trace=trace
    )
    LAST_RESULTS = res
    h_new = np.concatenate(
        [res.results[k]["h_out"][:VPC] for k in range(N_CORES)], axis=0
    )
    c_new = np.concatenate(
        [res.results[k]["c_out"][:VPC] for k in range(N_CORES)], axis=0
    )
    return h_new, c_new
